# revision 16
# baseline (speedup 1.0000x reference)
"""Zero-communication Trainium2 attention-head kernel (softmax over queries).

Every core computes the FULL softmax denominator (all scores + exp) locally
and the output rows of its own query slab only.  No cross-core traffic, so
each core's NEFF span is independent of launch stagger — the max-over-cores
exec time collapses to per-core compute.

Per core (inputs host-prepped bf16 hi/lo; q columns ROTATED so the core's
slab occupies i in [0, SLAB)):
  1. Stream + project q (3-term bf16, result duplicated into both PSUM
     halves via tile_position), split to stacked Q1=[qh;ql], Q2=[ql;qh].
     Same for k -> KS=[kh;kl].  v projected per j-chunk -> vt bf16.
  2. Per j-tile (128 keys): 8 score chunks [128, ICH] via 2 stacked
     matmuls each (all 4 hi/lo cross terms: KS^T Q1 = kh qh + kl ql,
     KS^T Q2 = kh ql + kl qh).  Chunks processed in 4 pairs; one DVE
     tensor_tensor_reduce per pair yields the negated pair-max (a valid
     exp bias: >= chunkmax-80, <= columnmax).  ACT exps every chunk with
     its pair bias, accumulating row sums; the slab chunk's exp values
     are kept in bf16.  Small per-tile combine -> colsum; 1/colsum and
     the slab pair's rescale weight are folded into vt.
  3. attn: ps_o[128, SLAB/2] (i halves stacked on partitions) accumulates
     vt-row x slab-exp matmuls over all NT tiles.
"""

import numpy as np

C = 8
QK = 64
VD = 64


def build_nc2(seq=8192, d=1024, reps=1, warmup=True):
    import concourse.bacc as bacc
    import concourse.mybir as mybir

    f32 = mybir.dt.float32
    bf16 = mybir.dt.bfloat16
    AX = mybir.AxisListType.X
    ALU = mybir.AluOpType
    ACTF = mybir.ActivationFunctionType

    NDT = d // 128
    SLAB = seq // C
    ICH = SLAB                   # score chunk width (i cols)
    NT = seq // 128              # j tiles
    QCH = 512                    # q stream/proj chunk (i cols)
    NQC = seq // QCH
    KCH = 512                    # k/v stream chunk (j cols)
    NKC = seq // KCH
    TPK = KCH // 128             # j tiles per k/v chunk
    QUN = NDT * QCH              # arena unit cols (one of hi/lo of a chunk)
    NDMA = NQC + NKC             # chunked input DMAs per iteration
    # chunk processed at position p: order 1..7 then 0 (slab last)
    POSC = [1, 2, 3, 4, 5, 6, 7, 0]
    INF = 3.0e38

    nc = bacc.Bacc(target_bir_lowering=False, debug=False)

    def din(name, w):
        return nc.declare_dram_parameter(name, [128, w], bf16, isOutput=False)

    q_d = din("qin", NQC * 2 * QUN)    # per chunk: [hi unit | lo unit]
    kv_d = din("kvin", NKC * 3 * QUN)  # per chunk: [kh | kl | v]
    wqh_d, wql_d = din("wqh", NDT * QK), din("wql", NDT * QK)
    wkh_d, wkl_d = din("wkh", NDT * QK), din("wkl", NDT * QK)
    wv_d = din("wv", NDT * VD)
    out_d = nc.declare_dram_parameter("out", [128, SLAB // 2], f32, isOutput=True)

    from contextlib import ExitStack

    with ExitStack() as ctx:
        block = ctx.enter_context(nc.Block())
        sem = lambda n: ctx.enter_context(nc.semaphore(n))
        sb = lambda n, shape, dt: ctx.enter_context(nc.sbuf_tensor(n, shape, dt))
        ps = lambda n, shape: ctx.enter_context(nc.psum_tensor(n, shape, f32))

        s_w = sem("s_w")          # 80/iter
        s_in = sem("s_in")        # NUNIT*16/iter
        s_qproj = sem("s_qproj")  # NQC/iter
        s_kproj = sem("s_kproj")  # NKC/iter
        s_vproj = sem("s_vproj")  # NKC/iter
        s_qsplit = sem("s_qsplit")  # NQC/iter
        s_ksplit = sem("s_ksplit")  # NKC/iter
        s_vtc = sem("s_vtc")      # NKC/iter
        s_sc = sem("s_sc")        # 8*NT/iter
        s_mx = sem("s_mx")        # 4*NT/iter
        s_ex = sem("s_ex")        # 8*NT/iter
        s_d = sem("s_d")          # NT/iter
        s_e2 = sem("s_e2")        # NT/iter
        s_vt2 = sem("s_vt2")      # NT/iter
        s_attn = sem("s_attn")    # NT/iter
        s_oc = sem("s_oc")        # 1/iter
        s_out = sem("s_out")      # 16/iter
        s_qcp = sem("s_qcp")      # 2*NQC/iter (q split copies)
        s_kcp = sem("s_kcp")      # 2*NKC/iter (k split copies)
        s_cmb = sem("s_cmb")      # 6*NT/iter (combine chain)

        arena_q = sb("arena_q", [128, 2 * 2 * QUN], bf16)
        arena_kv = sb("arena_kv", [128, 2 * 3 * QUN], bf16)
        Q1 = sb("Q1", [128, seq], bf16)
        Q2 = sb("Q2", [128, seq], bf16)
        KS = sb("KS", [128, seq], bf16)
        ktmp = sb("ktmp", [128, KCH], bf16)
        vt = sb("vt", [128, NT * VD], bf16)
        wqh = sb("wqh_s", [128, NDT * QK], bf16)
        wql = sb("wql_s", [128, NDT * QK], bf16)
        wkh = sb("wkh_s", [128, NDT * QK], bf16)
        wkl = sb("wkl_s", [128, NDT * QK], bf16)
        wv = sb("wv_s", [128, NDT * VD], bf16)
        slab_e = sb("slab_e", [128, 2 * ICH], bf16)
        scr = sb("scr", [128, 2 * ICH], bf16)  # ACT non-slab exp dump (x2)
        nmax = sb("nmax", [128, 5 * NT], f32)
        ssum = sb("ssum", [128, 8 * NT], f32)
        nB = sb("nB", [128, NT], f32)
        e_all = sb("e_all", [128, 5 * NT], f32)
        spair = sb("spair", [128, 3 * NT], f32)
        sw_all = sb("sw_all", [128, 3 * NT], f32)
        sw2 = sb("sw2", [128, 2 * NT], f32)
        cs2 = sb("cs2", [128, NT], f32)
        cs_all = sb("cs_all", [128, NT], f32)
        rS_all = sb("rS_all", [128, NT], f32)
        fS_all = sb("fS_all", [128, NT], f32)
        out_sb = sb("out_sb", [128, SLAB // 2], f32)

        ps_ab = ps("ps_ab", [128, max(2 * ICH, 4 * QCH)])
        ps_c = ps("ps_c", [128, max(ICH, 2 * QCH)])
        ps_o = ps("ps_o", [128, SLAB // 2])
        ps_v = ps("ps_v", [128, 512])

        def proj_slot(r):
            if r % 6 < 4:
                off = (r % 4) * QCH
                return ps_ab[:, off : off + QCH]
            off = (r % 2) * QCH
            return ps_c[:, off : off + QCH]

        def pos_slot(p):
            # positions 0..6 alternate ps_ab halves; 7 -> ps_c
            if p == 7:
                return ps_c[:, 0:ICH]
            return ps_ab[:, (p % 2) * ICH : (p % 2 + 1) * ICH]

        # ---------------- SYNC: DMAs (one dma per chunk, serialized) -----
        @block.sync
        def _(s):
          for it in range(reps):
            if it > 0:
                s.wait_ge(s_vproj, it * NKC)  # arenas fully consumed

            def dma(dst, src, sem_=s_in):
                s.dma_start(out=dst, in_=src).then_inc(sem_, 16)

            dma(wqh[:, :], wqh_d[:, :], s_w)
            dma(wql[:, :], wql_d[:, :], s_w)
            dma(wkh[:, :], wkh_d[:, :], s_w)
            dma(wkl[:, :], wkl_d[:, :], s_w)
            dma(wv[:, :], wv_d[:, :], s_w)

            n = 0
            for ic in range(NQC):
                if ic >= 2:
                    s.wait_ge(s_qproj, it * NQC + ic - 1)  # slot free
                dma(arena_q[:, (ic % 2) * 2 * QUN : (ic % 2 + 1) * 2 * QUN],
                    q_d[:, ic * 2 * QUN : (ic + 1) * 2 * QUN])
                n += 1
                # serialize so every s_in level is an ordered sync point
                s.wait_ge(s_in, it * NDMA * 16 + n * 16)
            for jc in range(NKC):
                if jc >= 2:
                    s.wait_ge(s_vproj, it * NKC + jc - 1)
                dma(arena_kv[:, (jc % 2) * 3 * QUN : (jc % 2 + 1) * 3 * QUN],
                    kv_d[:, jc * 3 * QUN : (jc + 1) * 3 * QUN])
                n += 1
                s.wait_ge(s_in, it * NDMA * 16 + n * 16)

            s.wait_ge(s_oc, it + 1)
            s.dma_start(out=out_d[:, :], in_=out_sb[:, :]).then_inc(s_out, 16)
            s.wait_ge(s_out, it * 16 + 16)

        # ---------------- TENSOR (PE) ----------------
        @block.tensor
        def _(t):
          for it in range(reps):
            for w_ in range(40 if (warmup and it == 0) else 0):
                t.matmul(
                    ps_ab[0:64, 0:QCH], Q1[:, 0:64], Q1[:, 512 : 512 + QCH],
                    start=(w_ == 0), stop=False,
                )
            t.wait_ge(s_w, it * 80 + 80)
            # q projection
            for ic in range(NQC):
                r = ic
                t.wait_ge(s_in, it * NDMA * 16 + (ic + 1) * 16)
                if r >= 6:
                    t.wait_ge(s_qsplit, it * NQC + (r - 6) + 1)
                sl = proj_slot(r)
                qb = (ic % 2) * 2 * QUN
                uh = arena_q[:, qb : qb + QUN]
                ul = arena_q[:, qb + QUN : qb + 2 * QUN]
                for pos in (0, 64):
                    for dd in range(NDT):
                        terms = ((wqh, uh), (wqh, ul), (wql, uh))
                        for ti, (W, X) in enumerate(terms):
                            mm = t.matmul(
                                sl[pos : pos + 64, :],
                                W[:, dd * QK : (dd + 1) * QK],
                                X[:, dd * QCH : (dd + 1) * QCH],
                                start=(dd == 0 and ti == 0),
                                stop=(dd == NDT - 1 and ti == 2),
                                tile_position=(0, pos),
                            )
                mm.then_inc(s_qproj, 1)
            # k + v projection
            for jc in range(NKC):
                r = NQC + jc
                t.wait_ge(s_in, it * NDMA * 16 + (NQC + jc + 1) * 16)
                pidx = r - 6
                if pidx >= 0:
                    if pidx < NQC:
                        t.wait_ge(s_qsplit, it * NQC + pidx + 1)
                    else:
                        t.wait_ge(s_ksplit, it * NKC + (pidx - NQC) + 1)
                sl = proj_slot(r)
                kb = (jc % 2) * 3 * QUN
                uh = arena_kv[:, kb : kb + QUN]
                ul = arena_kv[:, kb + QUN : kb + 2 * QUN]
                for pos in (0, 64):
                    for dd in range(NDT):
                        terms = ((wkh, uh), (wkh, ul), (wkl, uh))
                        for ti, (W, X) in enumerate(terms):
                            mm = t.matmul(
                                sl[pos : pos + 64, :],
                                W[:, dd * QK : (dd + 1) * QK],
                                X[:, dd * KCH : (dd + 1) * KCH],
                                start=(dd == 0 and ti == 0),
                                stop=(dd == NDT - 1 and ti == 2),
                                tile_position=(0, pos),
                            )
                mm.then_inc(s_kproj, 1)
                # v proj for this chunk's tiles -> ps_v half jc%2
                if jc >= 2:
                    t.wait_ge(s_vtc, it * NKC + jc - 1)
                uv = arena_kv[:, kb + 2 * QUN : kb + 3 * QUN]
                vh = (jc % 2) * 256
                for t4 in range(TPK):
                    for dd in range(NDT):
                        mm = t.matmul(
                            ps_v[:, vh + t4 * 64 : vh + (t4 + 1) * 64],
                            uv[:, dd * KCH + t4 * 128 : dd * KCH + t4 * 128 + 128],
                            wv[:, dd * VD : (dd + 1) * VD],
                            start=(dd == 0),
                            stop=(dd == NDT - 1),
                        )
                mm.then_inc(s_vproj, 1)

            # scores + attn (serial phase: all proj/splits done first)
            t.wait_ge(s_qsplit, it * NQC + NQC)
            t.wait_ge(s_ksplit, it * NKC + NKC)
            for tt in range(NT):
                kt = KS[:, tt * 128 : (tt + 1) * 128]
                for p in range(8):
                    ch = POSC[p]
                    P = pos_slot(p)
                    gp = it * 8 * NT + tt * 8 + p
                    prev = gp - 7 if p == 7 else gp - 2
                    if prev >= 0:
                        t.wait_ge(s_ex, prev + 1)
                    nh = max(1, ICH // 512)
                    w_i = ICH if ICH < 512 else 512
                    for hh in range(nh):
                        cA = ch * ICH + hh * w_i
                        t.matmul(
                            P[:, hh * w_i : (hh + 1) * w_i],
                            kt, Q1[:, cA : cA + w_i],
                            start=True, stop=False,
                        )
                        mm = t.matmul(
                            P[:, hh * w_i : (hh + 1) * w_i],
                            kt, Q2[:, cA : cA + w_i],
                            start=False, stop=True,
                        )
                    mm.then_inc(s_sc, 1)
                    if p == 4 and tt > 0:
                        # attn of tile tt-1
                        t.wait_ge(s_vt2, it * NT + tt)
                        sl_e = slab_e[
                            :, ((tt - 1) % 2) * ICH : ((tt - 1) % 2 + 1) * ICH
                        ]
                        vrow = vt[:, (tt - 1) * VD : tt * VD]
                        for pos in (0, 64):
                            mm2 = t.matmul(
                                ps_o[pos : pos + 64, :],
                                vrow,
                                sl_e[
                                    :, (pos // 64) * (ICH // 2)
                                    : (pos // 64 + 1) * (ICH // 2)
                                ],
                                start=(tt - 1 == 0),
                                stop=False,
                                tile_position=(0, pos),
                                skip_group_check=True,
                            )
                        mm2.then_inc(s_attn, 1)
            # last tile's attn
            t.wait_ge(s_vt2, it * NT + NT)
            sl_e = slab_e[:, ((NT - 1) % 2) * ICH : ((NT - 1) % 2 + 1) * ICH]
            vrow = vt[:, (NT - 1) * VD : NT * VD]
            for pos in (0, 64):
                mm2 = t.matmul(
                    ps_o[pos : pos + 64, :],
                    vrow,
                    sl_e[:, (pos // 64) * (ICH // 2) : (pos // 64 + 1) * (ICH // 2)],
                    start=(NT - 1 == 0),
                    stop=True,
                    tile_position=(0, pos),
                    skip_group_check=True,
                )
            mm2.then_inc(s_attn, 1)

        # ---------------- VECTOR (DVE) ----------------
        cmb_n = [0]

        def combine(v, it, tt):
            c3 = slice(tt * 3, tt * 3 + 3)
            v.wait_ge(s_ex, it * 8 * NT + tt * 8 + 8)
            v.wait_ge(s_e2, it * NT + tt + 1)
            s6 = ssum[:, tt * 8 : tt * 8 + 6].rearrange(
                "p (a two) -> p a two", a=3, two=2
            )

            def step(inst):
                inst.then_inc(s_cmb, 1)
                cmb_n[0] += 1

            def cw():
                v.wait_ge(s_cmb, cmb_n[0])

            # group sums: pairs (0,1) (2,3) (4,5), singles 6 and 7(slab)
            step(v.tensor_tensor(
                spair[:, c3], s6[:, :, 0], s6[:, :, 1], op=ALU.add
            ))
            step(v.tensor_tensor(
                sw2[:, tt * 2 : tt * 2 + 2],
                e_all[:, tt * 5 + 3 : tt * 5 + 5],
                ssum[:, tt * 8 + 6 : tt * 8 + 8], op=ALU.mult,
            ))
            cw()
            step(v.tensor_tensor(
                sw_all[:, c3], e_all[:, tt * 5 : tt * 5 + 3],
                spair[:, c3], op=ALU.mult,
            ))
            cw()
            step(v.tensor_reduce(
                cs_all[:, tt : tt + 1], sw_all[:, c3], axis=AX, op=ALU.add
            ))
            step(v.tensor_reduce(
                cs2[:, tt : tt + 1], sw2[:, tt * 2 : tt * 2 + 2],
                axis=AX, op=ALU.add,
            ))
            cw()
            step(v.tensor_tensor(
                cs_all[:, tt : tt + 1], cs_all[:, tt : tt + 1],
                cs2[:, tt : tt + 1], op=ALU.add,
            ))
            cw()
            step(v.reciprocal(rS_all[:, tt : tt + 1], cs_all[:, tt : tt + 1]))
            cw()
            step(v.tensor_tensor(
                fS_all[:, tt : tt + 1], e_all[:, tt * 5 + 4 : tt * 5 + 5],
                rS_all[:, tt : tt + 1], op=ALU.mult,
            ))
            cw()
            v.wait_ge(s_vtc, it * NKC + tt // TPK + 1)
            v.tensor_scalar_mul(
                vt[:, tt * VD : (tt + 1) * VD],
                vt[:, tt * VD : (tt + 1) * VD],
                fS_all[:, tt : tt + 1],
            ).then_inc(s_vt2, 1)

        @block.vector
        def _(v):
          for it in range(reps):
            # q splits
            for ic in range(NQC):
                v.wait_ge(s_qproj, it * NQC + ic + 1)
                sl = proj_slot(ic)
                cols = slice(ic * QCH, (ic + 1) * QCH)
                v.tensor_copy(Q1[0:64, cols], sl[0:64, :]).then_inc(s_qcp, 1)
                v.tensor_copy(Q2[64:128, cols], sl[64:128, :]).then_inc(s_qcp, 1)
                v.wait_ge(s_qcp, it * 2 * NQC + 2 * ic + 2)
                v.tensor_tensor(
                    Q1[64:128, cols], sl[64:128, :], Q2[64:128, cols],
                    op=ALU.subtract,
                )
                v.tensor_tensor(
                    Q2[0:64, cols], sl[0:64, :], Q1[0:64, cols],
                    op=ALU.subtract,
                ).then_inc(s_qsplit, 1)
            # k splits + v truncs
            for jc in range(NKC):
                v.wait_ge(s_kproj, it * NKC + jc + 1)
                if it * NKC + jc > 0:
                    v.wait_ge(s_ksplit, it * NKC + jc)  # ktmp reuse
                sl = proj_slot(NQC + jc)
                cols = slice(jc * KCH, (jc + 1) * KCH)
                v.tensor_copy(KS[0:64, cols], sl[0:64, :]).then_inc(s_kcp, 1)
                v.tensor_copy(ktmp[64:128, :], sl[64:128, :]).then_inc(s_kcp, 1)
                v.wait_ge(s_kcp, it * 2 * NKC + 2 * jc + 2)
                v.tensor_tensor(
                    KS[64:128, cols], sl[64:128, :], ktmp[64:128, :],
                    op=ALU.subtract,
                ).then_inc(s_ksplit, 1)
                v.wait_ge(s_vproj, it * NKC + jc + 1)
                vh = (jc % 2) * 256
                v.tensor_copy(
                    vt[:, jc * TPK * VD : (jc + 1) * TPK * VD],
                    ps_v[:, vh : vh + TPK * VD],
                ).then_inc(s_vtc, 1)

            for tt in range(NT):
                if tt > 0:
                    combine(v, it, tt - 1)
                for pr in range(3):
                    v.wait_ge(s_sc, it * 8 * NT + tt * 8 + 2 * pr + 2)
                    v.tensor_reduce(
                        nmax[:, tt * 5 + pr : tt * 5 + pr + 1],
                        ps_ab[:, 0 : 2 * ICH], axis=AX, op=ALU.max,
                        negate=True,
                    ).then_inc(s_mx, 1)
                v.wait_ge(s_sc, it * 8 * NT + tt * 8 + 7)
                v.tensor_reduce(
                    nmax[:, tt * 5 + 3 : tt * 5 + 4],
                    ps_ab[:, 0:ICH], axis=AX, op=ALU.max, negate=True,
                ).then_inc(s_mx, 1)
                v.wait_ge(s_sc, it * 8 * NT + tt * 8 + 8)
                v.tensor_reduce(
                    nmax[:, tt * 5 + 4 : tt * 5 + 5],
                    ps_c[:, 0:ICH], axis=AX, op=ALU.max, negate=True,
                ).then_inc(s_mx, 1)
                v.wait_ge(s_mx, it * 5 * NT + tt * 5 + 5)
                v.tensor_reduce(
                    nB[:, tt : tt + 1], nmax[:, tt * 5 : tt * 5 + 5],
                    axis=AX, op=ALU.min,
                ).then_inc(s_d, 1)
            combine(v, it, NT - 1)
            v.wait_ge(s_attn, it * NT + NT)
            v.tensor_copy(out_sb[:, :], ps_o[:, :]).then_inc(s_oc, 1)

        # ---------------- SCALAR (ACT) ----------------
        @block.scalar
        def _(sc):
          for it in range(reps):
            for tt in range(NT):
                for p in range(8):
                    P = pos_slot(p)
                    grp = p // 2 if p < 6 else p - 3
                    sc.wait_ge(s_mx, it * 5 * NT + tt * 5 + grp + 1)
                    bias = nmax[:, tt * 5 + grp : tt * 5 + grp + 1]
                    acc = ssum[:, tt * 8 + p : tt * 8 + p + 1]
                    if p == 7:
                        if it * NT + tt - 1 > 0:
                            sc.wait_ge(s_attn, it * NT + tt - 1)
                        outap = slab_e[:, (tt % 2) * ICH : (tt % 2 + 1) * ICH]
                    else:
                        outap = scr[:, (p % 2) * ICH : (p % 2 + 1) * ICH]
                    sc.activation(
                        outap, P, ACTF.Exp,
                        bias=bias, scale=1.0, accum_out=acc,
                    ).then_inc(s_ex, 1)
                # pair rescale weights e = exp(nB - nmax)
                sc.wait_ge(s_d, it * NT + tt + 1)
                sc.activation(
                    e_all[:, tt * 5 : tt * 5 + 5],
                    nmax[:, tt * 5 : tt * 5 + 5],
                    ACTF.Exp, scale=-1.0, bias=nB[:, tt : tt + 1],
                ).then_inc(s_e2, 1)

    nc.finalize()
    return nc


# ------------------------- host side -------------------------

def _split_bf16(x):
    import ml_dtypes

    hi = x.astype(ml_dtypes.bfloat16)
    lo = (x - hi.astype(np.float32)).astype(ml_dtypes.bfloat16)
    return hi, lo


def _tile_cols(xT, w):
    dd = xT.shape[0] // 128
    return np.ascontiguousarray(
        xT.reshape(dd, 128, w).transpose(1, 0, 2).reshape(128, dd * w)
    )


def _tile_chunked(xT, ch):
    """[d, s] -> [128, (s/ch)*(d/128)*ch], col = jc*(nd*ch) + dd*ch + jj."""
    d, s = xT.shape
    nd = d // 128
    njc = s // ch
    a = xT.reshape(nd, 128, njc, ch).transpose(1, 2, 0, 3)
    return np.ascontiguousarray(a.reshape(128, njc * nd * ch))


def build_in_maps2(inputs, seq=8192, d=1024):
    import ml_dtypes

    bf = ml_dtypes.bfloat16
    SLAB = seq // C
    QCH = 512
    KCH = 512
    qw8 = (inputs["query_weights"] / np.sqrt(np.float32(QK))).astype(np.float32)
    wqh, wql = _split_bf16(qw8)
    wkh, wkl = _split_bf16(inputs["key_weights"].astype(np.float32))
    wv = inputs["value_weights"].astype(bf)
    w_tiled = {
        "wqh": _tile_cols(wqh.astype(np.float32), QK).astype(bf),
        "wql": _tile_cols(wql.astype(np.float32), QK).astype(bf),
        "wkh": _tile_cols(wkh.astype(np.float32), QK).astype(bf),
        "wkl": _tile_cols(wkl.astype(np.float32), QK).astype(bf),
        "wv": _tile_cols(wv.astype(np.float32), VD).astype(bf),
    }
    nd = d // 128
    NQC = seq // QCH
    NKC = seq // KCH
    QUN = nd * QCH

    def interleave(parts, nch):
        """each part [128, nch*QUN] chunk-major -> [128, nch*len(parts)*QUN]
        with chunk ic holding [part0 | part1 | ...]."""
        stacked = np.concatenate(
            [p.reshape(128, nch, QUN) for p in parts], axis=2
        )
        return np.ascontiguousarray(
            stacked.reshape(128, nch * len(parts) * QUN)
        )

    kT = np.ascontiguousarray(inputs["keys"].T).astype(np.float32)
    vT = np.ascontiguousarray(inputs["values"].T).astype(np.float32)
    kh, kl = _split_bf16(kT)
    kv_int = interleave(
        [
            _tile_chunked(kh.astype(np.float32), KCH).astype(bf),
            _tile_chunked(kl.astype(np.float32), KCH).astype(bf),
            _tile_chunked(vT.astype(np.float32), KCH).astype(bf),
        ],
        NKC,
    )
    qT = np.ascontiguousarray(inputs["queries"].T).astype(np.float32)
    in_maps = []
    for c in range(C):
        qrot = np.roll(qT, -c * SLAB, axis=1)
        qh, ql = _split_bf16(qrot)
        q_int = interleave(
            [
                _tile_chunked(qh.astype(np.float32), QCH).astype(bf),
                _tile_chunked(ql.astype(np.float32), QCH).astype(bf),
            ],
            NQC,
        )
        m = {"qin": q_int, "kvin": kv_int}
        m.update(w_tiled)
        in_maps.append(m)
    return in_maps


def assemble_out2(results, seq=8192):
    SLAB = seq // C
    full = np.zeros((seq, VD), np.float32)
    for c in range(C):
        o = np.asarray(results[c]["out"], dtype=np.float32)
        slab = np.concatenate([o[0:64, :], o[64:128, :]], axis=1).T
        full[c * SLAB : (c + 1) * SLAB] = slab
    return full




def run_spmd_staged(nc, in_maps, profile_dir=None):
    """run_bass_via_pjrt with inputs pre-staged on-device (blocks until all
    shards are resident) so the 8 cores launch aligned instead of staggered
    by per-device input-transfer time. Optionally wraps the execute in the
    axon NTFF profile hook (profile_dir)."""
    import jax
    import numpy as np_
    from jax.sharding import Mesh, PartitionSpec, NamedSharding
    from jax.experimental.shard_map import shard_map
    import concourse.mybir as mybir
    from concourse import bass2jax

    bass2jax.install_neuronx_cc_hook()
    n_cores = len(in_maps)

    partition_name = (
        nc.partition_id_tensor.name if nc.partition_id_tensor else None
    )
    in_names, out_names, out_avals, zero_outs = [], [], [], []
    for alloc in nc.m.functions[0].allocations:
        if not isinstance(alloc, mybir.MemoryLocationSet):
            continue
        name = alloc.memorylocations[0].name
        if alloc.kind == "ExternalInput":
            if name != partition_name:
                in_names.append(name)
        elif alloc.kind == "ExternalOutput":
            out_names.append(name)
            shape = tuple(alloc.tensor_shape)
            dtype = mybir.dt.np(alloc.dtype)
            out_avals.append(jax.core.ShapedArray(shape, dtype))
            zero_outs.append(np_.zeros(shape, dtype))
    n_params = len(in_names)
    n_outs = len(out_avals)
    all_names = in_names + out_names
    if partition_name is not None:
        all_names = all_names + [partition_name]

    def _body(*args):
        operands = list(args)
        if partition_name is not None:
            operands.append(bass2jax.partition_id_tensor())
        outs = bass2jax._bass_exec_p.bind(
            *operands,
            out_avals=tuple(out_avals),
            in_names=tuple(all_names),
            out_names=tuple(out_names),
            lowering_input_output_aliases=(),
            sim_require_finite=True,
            sim_require_nnan=True,
            nc=nc,
        )
        return tuple(outs)

    devices = jax.devices()[:n_cores]
    mesh = Mesh(np_.asarray(devices), ("core",))
    spec = NamedSharding(mesh, PartitionSpec("core"))
    sharded = jax.jit(
        shard_map(
            _body,
            mesh=mesh,
            in_specs=(PartitionSpec("core"),) * (n_params + n_outs),
            out_specs=(PartitionSpec("core"),) * n_outs,
            check_rep=False,
        ),
        donate_argnums=tuple(range(n_params, n_params + n_outs)),
        keep_unused=True,
    )
    concat_in = [
        np_.concatenate([np_.asarray(in_maps[c][nm]) for c in range(n_cores)], axis=0)
        for nm in in_names
    ]
    concat_zero = [
        np_.zeros((n_cores * z.shape[0], *z.shape[1:]), z.dtype) for z in zero_outs
    ]
    staged = [jax.device_put(a, spec) for a in concat_in + concat_zero]
    jax.block_until_ready(staged)

    if profile_dir is not None:
        from antenv.axon_hooks import get_axon_ntff_profile_hook

        hook = get_axon_ntff_profile_hook()
        with hook(profile_dir, list(range(n_cores))):
            out_arrs = sharded(*staged)
            jax.block_until_ready(out_arrs)
    else:
        out_arrs = sharded(*staged)
    return [
        {
            nm: np_.asarray(out_arrs[i]).reshape(n_cores, *out_avals[i].shape)[c]
            for i, nm in enumerate(out_names)
        }
        for c in range(n_cores)
    ]




def kernel(queries, keys, values, query_weights, key_weights, value_weights):
    import sys

    for p in ("/opt/trn_rl_repo",):
        if p not in sys.path:
            sys.path.insert(0, p)

    seq, d = queries.shape
    inputs = {
        "queries": queries, "keys": keys, "values": values,
        "query_weights": query_weights, "key_weights": key_weights,
        "value_weights": value_weights,
    }
    in_maps = build_in_maps2(inputs, seq=seq, d=d)
    nc = build_nc2(seq=seq, d=d)
    results = run_spmd_staged(nc, in_maps)
    return assemble_out2(results, seq=seq)
